# revision 12
# baseline (speedup 1.0000x reference)
"""Trainium2 Bass kernel for MultiHeadAttention (B=4, S=2048, D=1024, H=16).

Sharding (8 cores): core c = (batch b=c//2, head-group g=c%2).
Each core handles 1 batch x 8 heads (proj dims g*512..(g+1)*512).

Per-core device program (all matmuls fp32r):
  - Projections from host-pre-transposed inputs:
      QT[pd, tok] = wq_t.T @ xq_t      (pd = 512 proj dims, tok = 2048)
      KT[pd, tok] = wk_t.T @ xk_t
      V[tok, vd]  = xv_t.T @ wv_t      (token-major, augmented w/ ones col/head)
  - Attention per head h (depth 64), scores computed TRANSPOSED:
      S'[k, q] = KT_h.T @ QT_h ;  E = exp(S'/8)  (softmax max-sub skipped: |S| small)
      rawT[d', q] = V_aug_h.T @ E   -> row 64 is the softmax denominator
      OT[d', q] = rawT[:64] * (1/denom)  (batched recip + partition-bcast + DVE mul)
  - Dense partial: out[tok, n] = OT.T @ ds_t  (ds_t = dense_w[:, g-slice].T)
Host: out[b] = partial[2b] + partial[2b+1] + dense_b.

Perf notes: fp32r matmuls self-load weights (~194ns per lhsT change), so MMs
sharing a stationary operand are emitted back-to-back; the two heads of a
partition-tile run QK row-packed (rows 0-63 / 64-127, concurrent via the PE
reorder window); phase emission order (K, Q-half, attention, V, ...) lets the
greedy scheduler fill PE stalls with projection work.

Self-contained: hardcodes shapes; builds/compiles the Bass program once per
process and reuses it.
"""

import numpy as np
from contextlib import ExitStack

import concourse.bass as bass
import concourse.tile as tile
from concourse import bacc, mybir
from concourse.bass_utils import run_bass_kernel_spmd

F32 = mybir.dt.float32
F32R = mybir.dt.float32r
EXP = mybir.ActivationFunctionType.Exp

P = 128
S = 2048          # tokens per batch
DM = 1024         # d_model
PD = 512          # proj dims per core (8 heads x 64)
NDC = DM // P     # 8 d_model chunks
NPT = PD // P     # 4 proj partition tiles
NTT = 16          # token tiles (128)
NH = 8            # heads per core
DEP = 64          # head depth
VW = NH * (DEP + 1)   # V tile width with ones-augmentation (520)


def _r(ap):
    return ap.bitcast(F32R)


def _emit(nc, tc, ctx, d, with_bias):
    has_qb, has_kb, has_vb = with_bias

    res = ctx.enter_context(tc.tile_pool(name="res", bufs=1))
    xw = ctx.enter_context(tc.tile_pool(name="xw", bufs=9))
    otp = ctx.enter_context(tc.tile_pool(name="otp", bufs=8))
    wpool = ctx.enter_context(tc.tile_pool(name="w", bufs=1))
    espool = ctx.enter_context(tc.tile_pool(name="es", bufs=3))
    bcpool = ctx.enter_context(tc.tile_pool(name="bc", bufs=3))
    rcpool = ctx.enter_context(tc.tile_pool(name="rc", bufs=1))
    outpool = ctx.enter_context(tc.tile_pool(name="osb", bufs=2))
    ps = ctx.enter_context(tc.tile_pool(name="ps", bufs=2, space="PSUM"))
    pvps = ctx.enter_context(tc.tile_pool(name="pvps", bufs=4, space="PSUM"))

    # ---- resident tiles -------------------------------------------------
    QT = [res.tile([P, S], F32R, name=f"QT{pt}", tag=f"QT{pt}") for pt in range(NPT)]
    KT = [res.tile([P, S], F32R, name=f"KT{pt}", tag=f"KT{pt}") for pt in range(NPT)]
    V = [res.tile([P, VW], F32R, name=f"V{tt}", tag=f"V{tt}") for tt in range(NTT)]

    bias_sb = {}
    if has_qb or has_kb or has_vb:
        ones_row = res.tile([1, 1024], F32R, name="ones_row", tag="ones_row")
        nc.sync.dma_start(out=ones_row, in_=_r(d["ones_r"][:, :]))
        for flag, nm in ((has_qb, "bq"), (has_kb, "bk"), (has_vb, "bv")):
            if flag:
                bias_sb[nm] = res.tile([1, PD], F32R, name=f"{nm}_sb", tag=f"{nm}_sb")
                nc.sync.dma_start(out=bias_sb[nm], in_=_r(d[nm][:, :]))

    def load_w(wname, half=None):
        """Load 8 weight chunks [128, 512] into the (reused) w0..w7 slots."""
        w_ch = []
        sfx = "" if half is None else f"_{half}"
        for dc in range(NDC):
            wt = wpool.tile([P, PD], F32R, name=f"{wname}{sfx}_{dc}", tag=f"w{dc}")
            nc.sync.dma_start(out=wt, in_=_r(d[wname][dc * P : (dc + 1) * P, :]))
            w_ch.append(wt)
        return w_ch

    def proj_qk(w_ch, xname, bname, out_tiles, th):
        """Project out_tiles[:, th*1024 ...] (1024 tokens)."""
        has_b = bname in bias_sb
        x_ch = []
        for dc in range(NDC):
            xt = xw.tile([P, 1024], F32R, name=f"x{xname}{th}_{dc}", tag="xt")
            nc.sync.dma_start(
                out=xt,
                in_=_r(d[xname][dc * P : (dc + 1) * P, th * 1024 : (th + 1) * 1024]),
            )
            x_ch.append(xt)
        for pt in range(NPT):
            pst = ps.tile([P, 1024], F32, name=f"ps{xname}{th}_{pt}", tag="qk")
            for dc in range(NDC):
                lhsT = _r(w_ch[dc][:, pt * P : (pt + 1) * P])
                for qs in range(2):
                    nc.tensor.matmul(
                        pst[:, qs * 512 : (qs + 1) * 512],
                        lhsT,
                        _r(x_ch[dc][:, qs * 512 : (qs + 1) * 512]),
                        start=(dc == 0),
                        stop=(dc == NDC - 1 and not has_b),
                    )
            if has_b:
                for qs in range(2):
                    nc.tensor.matmul(
                        pst[:, qs * 512 : (qs + 1) * 512],
                        _r(bias_sb[bname][:, pt * P : (pt + 1) * P]),
                        _r(ones_row[:, qs * 512 : (qs + 1) * 512]),
                        start=False,
                        stop=True,
                    )
            nc.vector.tensor_copy(
                out_tiles[pt][:, th * 1024 : (th + 1) * 1024], pst
            )

    def proj_v(wv_ch, th):
        """V token tiles tt = th*8 .. th*8+7."""
        x_ch = []
        for dc in range(NDC):
            xt = xw.tile([P, 1024], F32R, name=f"xv{th}_{dc}", tag="xt")
            nc.sync.dma_start(
                out=xt,
                in_=_r(d["xv_t"][dc * P : (dc + 1) * P, th * 1024 : (th + 1) * 1024]),
            )
            x_ch.append(xt)
        for tb in range(8):
            tt = th * 8 + tb
            pst = ps.tile([P, 1024], F32, name=f"psv{tt}", tag="qk")[:, :512]
            for dc in range(NDC):
                nc.tensor.matmul(
                    pst,
                    _r(x_ch[dc][:, tb * P : (tb + 1) * P]),
                    _r(wv_ch[dc]),
                    start=(dc == 0),
                    stop=(dc == NDC - 1 and not has_vb),
                )
            if has_vb:
                nc.tensor.matmul(
                    pst, _r(ones_row[:, :P]), _r(bias_sb["bv"]), start=False, stop=True
                )
            v3 = V[tt].rearrange("p (h c) -> p h c", c=DEP + 1)
            nc.vector.tensor_copy(v3[:, :, 0:DEP], pst)
            # ones column (col h*65+64) from a DRAM constant
            nc.sync.dma_start(out=v3[:, :, DEP : DEP + 1], in_=_r(d["ones_c"][:, :]))

    def attention(hp, qb, OT_tile):
        """Heads A=2hp (rows 0-63), B=2hp+1 (rows 64-127); queries
        qb*1024..(qb+1)*1024. Writes OT_tile[128, 1024]."""
        A, B = 2 * hp, 2 * hp + 1
        halves = ((A, 0), (B, DEP))
        pv = {}
        for hh, _ in halves:
            for qs in range(2):
                pv[(hh, qs)] = pvps.tile(
                    [DEP + 1, 512], F32, name=f"pv{qb}_{hh}_{qs}", tag="pv"
                )
        for kt in range(NTT):
            qk = {
                hh: ps.tile([P, 1024], F32, name=f"qk{qb}_{hh}_{kt}", tag="qk")
                for hh, _ in halves
            }
            for hh, r0 in halves:
                lhsT = _r(KT[hp][r0 : r0 + DEP, kt * P : (kt + 1) * P])
                for qs in range(2):
                    q0 = qb * 1024 + qs * 512
                    nc.tensor.matmul(
                        qk[hh][:, qs * 512 : (qs + 1) * 512],
                        lhsT,
                        _r(QT[hp][r0 : r0 + DEP, q0 : q0 + 512]),
                        start=True,
                        stop=True,
                    )
            es = {}
            for hh, _ in halves:
                es[hh] = espool.tile(
                    [P, 1024], F32R, name=f"es{qb}_{hh}_{kt}", tag="es"
                )
                nc.scalar.activation(es[hh], qk[hh], EXP, scale=0.125)
            for hh, _ in halves:
                lhsT = _r(V[kt][:, hh * (DEP + 1) : (hh + 1) * (DEP + 1)])
                for qs in range(2):
                    nc.tensor.matmul(
                        pv[(hh, qs)],
                        lhsT,
                        _r(es[hh][:, qs * 512 : (qs + 1) * 512]),
                        start=(kt == 0),
                        stop=(kt == NTT - 1),
                    )
        # finalize: batched reciprocal of the 4 denominator rows (placed at
        # partitions 0/32/64/96 -- other bases are rejected)
        order = [(A, 0), (A, 1), (B, 0), (B, 1)]
        den = rcpool.tile([128, 512], F32, name=f"den{qb}_{hp}", tag="den")
        nc.vector.memset(den, 1.0)
        for j, (hh, qs) in enumerate(order):
            nc.vector.tensor_copy(
                den[32 * j : 32 * j + 1, :], pv[(hh, qs)][DEP : DEP + 1, :]
            )
        rec = rcpool.tile([128, 512], F32, name=f"rec{qb}_{hp}", tag="rec")
        nc.vector.reciprocal(rec, den)
        for j, (hh, qs) in enumerate(order):
            bct = bcpool.tile([DEP, 512], F32, name=f"bct{qb}_{hh}_{qs}", tag="bct")
            nc.gpsimd.partition_broadcast(bct, rec[32 * j : 32 * j + 1, :])
            r0 = (hh % 2) * DEP
            nc.vector.tensor_mul(
                OT_tile[r0 : r0 + DEP, qs * 512 : (qs + 1) * 512],
                pv[(hh, qs)][0:DEP, :],
                bct,
            )

    # ---------------- emission (priority) order --------------------------
    # K proj (all tokens) -> Q proj half 0 -> attention qb=0 -> V proj ->
    # Q proj half 1 -> attention qb=1 -> dense.  Attention qb=0 is emitted
    # before V-proj/Q-h1 only in the sense that V/Q-h1 land after it in
    # priority; tile-granular deps let the scheduler fill PE stalls.
    OT_all = {}

    wk_ch = load_w("wk_t")
    for th in range(2):
        proj_qk(wk_ch, "xk_t", "bk", KT, th)
    wq_ch = load_w("wq_t", 0)
    proj_qk(wq_ch, "xq_t", "bq", QT, 0)

    wv_ch = load_w("wv_t")
    for th in range(2):
        proj_v(wv_ch, th)

    for hp in range(NPT):
        OT_all[(hp, 0)] = otp.tile([P, 1024], F32R, name=f"OT_{hp}_0", tag="ot")
        attention(hp, 0, OT_all[(hp, 0)])

    wq_ch = load_w("wq_t", 1)
    proj_qk(wq_ch, "xq_t", "bq", QT, 1)

    for hp in range(NPT):
        OT_all[(hp, 1)] = otp.tile([P, 1024], F32R, name=f"OT_{hp}_1", tag="ot")
        attention(hp, 1, OT_all[(hp, 1)])

    # ---- dense ----------------------------------------------------------
    ds_ch = {}
    for pt in range(NPT):
        for nt in range(2):
            wt = wpool.tile(
                [P, PD], F32R, name=f"ds{pt}_{nt}", tag=f"w{pt * 2 + nt}"
            )
            nc.sync.dma_start(
                out=wt,
                in_=_r(d["ds_t"][pt * P : (pt + 1) * P, nt * 512 : (nt + 1) * 512]),
            )
            ds_ch[(pt, nt)] = wt

    for tt in range(NTT):
        qh, tb = tt // 8, tt % 8
        pst = ps.tile([P, 1024], F32, name=f"psd{tt}", tag="qk")
        for pt in range(NPT):
            lhsT = _r(OT_all[(pt, qh)][:, tb * P : (tb + 1) * P])
            for nt in range(2):
                nc.tensor.matmul(
                    pst[:, nt * 512 : (nt + 1) * 512],
                    lhsT,
                    _r(ds_ch[(pt, nt)]),
                    start=(pt == 0),
                    stop=(pt == NPT - 1),
                )
        for nt in range(2):
            osb = outpool.tile([P, 512], F32, name=f"osb{tt}_{nt}", tag="osb")
            nc.vector.tensor_copy(osb, pst[:, nt * 512 : (nt + 1) * 512])
            nc.sync.dma_start(
                out=d["out"][tt * P : (tt + 1) * P, nt * 512 : (nt + 1) * 512],
                in_=osb,
            )


def build_nc(with_bias=(False, False, False)):
    nc = bacc.Bacc("TRN2", target_bir_lowering=False, debug=False)
    d = {}
    for name, shape in (
        ("xq_t", [DM, S]),
        ("xk_t", [DM, S]),
        ("xv_t", [DM, S]),
        ("wq_t", [DM, PD]),
        ("wk_t", [DM, PD]),
        ("wv_t", [DM, PD]),
        ("ds_t", [PD, DM]),
        ("ones_c", [P, NH]),
    ):
        d[name] = nc.dram_tensor(name, shape, F32, kind="ExternalInput").ap()
    if any(with_bias):
        d["ones_r"] = nc.dram_tensor(
            "ones_r", [1, 1024], F32, kind="ExternalInput"
        ).ap()
    for flag, nm in zip(with_bias, ("bq", "bk", "bv")):
        if flag:
            d[nm] = nc.dram_tensor(nm, [1, PD], F32, kind="ExternalInput").ap()
    d["out"] = nc.dram_tensor("out", [S, DM], F32, kind="ExternalOutput").ap()

    with tile.TileContext(nc) as tc:
        with ExitStack() as ctx:
            _emit(nc, tc, ctx, d, with_bias)
    nc.compile()
    return nc


_CACHE = {}


def _get_nc(with_bias):
    if with_bias not in _CACHE:
        _CACHE[with_bias] = build_nc(with_bias)
    return _CACHE[with_bias]


def make_in_maps(query, key, value, wq_w, wq_b, wk_w, wk_b, wv_w, wv_b, dense_w):
    """Host-side sharding: 8 in_maps for cores (b, g)."""
    with_bias = (
        bool(np.any(wq_b)),
        bool(np.any(wk_b)),
        bool(np.any(wv_b)),
    )
    c = np.ascontiguousarray
    in_maps = []
    for core in range(8):
        b, g = core // 2, core % 2
        sl = slice(g * PD, (g + 1) * PD)
        m = {
            "xq_t": c(query[b].T),
            "xk_t": c(key[b].T),
            "xv_t": c(value[b].T),
            "wq_t": c(wq_w[sl].T),
            "wk_t": c(wk_w[sl].T),
            "wv_t": c(wv_w[sl].T),
            "ds_t": c(dense_w[:, sl].T),
            "ones_c": np.ones((P, NH), np.float32),
        }
        if any(with_bias):
            m["ones_r"] = np.ones((1, 1024), np.float32)
        if with_bias[0]:
            m["bq"] = c(wq_b[sl][None, :])
        if with_bias[1]:
            m["bk"] = c(wk_b[sl][None, :])
        if with_bias[2]:
            m["bv"] = c(wv_b[sl][None, :])
        in_maps.append(m)
    return in_maps, with_bias


def kernel(
    query, key, value, wq_w, wq_b, wk_w, wk_b, wv_w, wv_b, dense_w, dense_b, **kw
):
    query = np.asarray(query, np.float32)
    key = np.asarray(key, np.float32)
    value = np.asarray(value, np.float32)
    in_maps, with_bias = make_in_maps(
        query, key, value,
        np.asarray(wq_w, np.float32), np.asarray(wq_b, np.float32),
        np.asarray(wk_w, np.float32), np.asarray(wk_b, np.float32),
        np.asarray(wv_w, np.float32), np.asarray(wv_b, np.float32),
        np.asarray(dense_w, np.float32),
    )
    nc = _get_nc(with_bias)
    res = run_bass_kernel_spmd(nc, in_maps, core_ids=list(range(8)))
    B = query.shape[0]
    out = np.empty((B, S, DM), np.float32)
    db = np.asarray(dense_b, np.float32)
    for b in range(B):
        out[b] = res.results[2 * b]["out"] + res.results[2 * b + 1]["out"] + db
    return out


# revision 13
# speedup vs baseline: 1.0017x; 1.0017x over previous
"""Trainium2 Bass kernel for MultiHeadAttention (B=4, S=2048, D=1024, H=16).

Sharding (8 cores): core c = (batch b=c//2, head-group g=c%2).
Each core handles 1 batch x 8 heads (proj dims g*512..(g+1)*512).

Per-core device program (all matmuls fp32r):
  - Projections from host-pre-transposed inputs:
      QT[pd, tok] = wq_t.T @ xq_t      (pd = 512 proj dims, tok = 2048)
      KT[pd, tok] = wk_t.T @ xk_t
      V[tok, vd]  = xv_t.T @ wv_t      (token-major, augmented w/ ones col/head)
  - Attention per head h (depth 64), scores computed TRANSPOSED:
      S'[k, q] = KT_h.T @ QT_h ;  E = exp(S'/8)  (softmax max-sub skipped: |S| small)
      rawT[d', q] = V_aug_h.T @ E   -> row 64 is the softmax denominator
      OT[d', q] = rawT[:64] * (1/denom)  (batched recip + partition-bcast + DVE mul)
  - Dense partial: out[tok, n] = OT.T @ ds_t  (ds_t = dense_w[:, g-slice].T)
Host: out[b] = partial[2b] + partial[2b+1] + dense_b.

Perf notes: fp32r matmuls self-load weights (~194ns per lhsT change), so MMs
sharing a stationary operand are emitted back-to-back; the two heads of a
partition-tile run QK row-packed (rows 0-63 / 64-127, concurrent via the PE
reorder window); phase emission order (K, Q-half, attention, V, ...) lets the
greedy scheduler fill PE stalls with projection work.

Self-contained: hardcodes shapes; builds/compiles the Bass program once per
process and reuses it.
"""

import numpy as np
from contextlib import ExitStack

import concourse.bass as bass
import concourse.tile as tile
from concourse import bacc, mybir
from concourse.bass_utils import run_bass_kernel_spmd

F32 = mybir.dt.float32
F32R = mybir.dt.float32r
EXP = mybir.ActivationFunctionType.Exp

P = 128
S = 2048          # tokens per batch
DM = 1024         # d_model
PD = 512          # proj dims per core (8 heads x 64)
NDC = DM // P     # 8 d_model chunks
NPT = PD // P     # 4 proj partition tiles
NTT = 16          # token tiles (128)
NH = 8            # heads per core
DEP = 64          # head depth
VW = NH * (DEP + 1)   # V tile width with ones-augmentation (520)


def _r(ap):
    return ap.bitcast(F32R)


def _emit(nc, tc, ctx, d, with_bias):
    has_qb, has_kb, has_vb = with_bias

    res = ctx.enter_context(tc.tile_pool(name="res", bufs=1))
    xw = ctx.enter_context(tc.tile_pool(name="xw", bufs=9))
    otp = ctx.enter_context(tc.tile_pool(name="otp", bufs=8))
    wpool = ctx.enter_context(tc.tile_pool(name="w", bufs=1))
    espool = ctx.enter_context(tc.tile_pool(name="es", bufs=3))
    bcpool = ctx.enter_context(tc.tile_pool(name="bc", bufs=3))
    rcpool = ctx.enter_context(tc.tile_pool(name="rc", bufs=1))
    outpool = ctx.enter_context(tc.tile_pool(name="osb", bufs=2))
    attn_psum_ctx = ExitStack()
    ps = attn_psum_ctx.enter_context(tc.tile_pool(name="ps", bufs=2, space="PSUM"))
    pvps = attn_psum_ctx.enter_context(
        tc.tile_pool(name="pvps", bufs=4, space="PSUM")
    )

    # ---- resident tiles -------------------------------------------------
    QT = [res.tile([P, S], F32R, name=f"QT{pt}", tag=f"QT{pt}") for pt in range(NPT)]
    KT = [res.tile([P, S], F32R, name=f"KT{pt}", tag=f"KT{pt}") for pt in range(NPT)]
    V = [res.tile([P, VW], F32R, name=f"V{tt}", tag=f"V{tt}") for tt in range(NTT)]

    bias_sb = {}
    if has_qb or has_kb or has_vb:
        ones_row = res.tile([1, 1024], F32R, name="ones_row", tag="ones_row")
        nc.sync.dma_start(out=ones_row, in_=_r(d["ones_r"][:, :]))
        for flag, nm in ((has_qb, "bq"), (has_kb, "bk"), (has_vb, "bv")):
            if flag:
                bias_sb[nm] = res.tile([1, PD], F32R, name=f"{nm}_sb", tag=f"{nm}_sb")
                nc.sync.dma_start(out=bias_sb[nm], in_=_r(d[nm][:, :]))

    def load_w(wname, half=None):
        """Load 8 weight chunks [128, 512] into the (reused) w0..w7 slots."""
        w_ch = []
        sfx = "" if half is None else f"_{half}"
        for dc in range(NDC):
            wt = wpool.tile([P, PD], F32R, name=f"{wname}{sfx}_{dc}", tag=f"w{dc}")
            nc.sync.dma_start(out=wt, in_=_r(d[wname][dc * P : (dc + 1) * P, :]))
            w_ch.append(wt)
        return w_ch

    def proj_qk(w_ch, xname, bname, out_tiles, th):
        """Project out_tiles[:, th*1024 ...] (1024 tokens)."""
        has_b = bname in bias_sb
        x_ch = []
        for dc in range(NDC):
            xt = xw.tile([P, 1024], F32R, name=f"x{xname}{th}_{dc}", tag="xt")
            nc.sync.dma_start(
                out=xt,
                in_=_r(d[xname][dc * P : (dc + 1) * P, th * 1024 : (th + 1) * 1024]),
            )
            x_ch.append(xt)
        for pt in range(NPT):
            pst = ps.tile([P, 1024], F32, name=f"ps{xname}{th}_{pt}", tag="qk")
            for dc in range(NDC):
                lhsT = _r(w_ch[dc][:, pt * P : (pt + 1) * P])
                for qs in range(2):
                    nc.tensor.matmul(
                        pst[:, qs * 512 : (qs + 1) * 512],
                        lhsT,
                        _r(x_ch[dc][:, qs * 512 : (qs + 1) * 512]),
                        start=(dc == 0),
                        stop=(dc == NDC - 1 and not has_b),
                    )
            if has_b:
                for qs in range(2):
                    nc.tensor.matmul(
                        pst[:, qs * 512 : (qs + 1) * 512],
                        _r(bias_sb[bname][:, pt * P : (pt + 1) * P]),
                        _r(ones_row[:, qs * 512 : (qs + 1) * 512]),
                        start=False,
                        stop=True,
                    )
            nc.vector.tensor_copy(
                out_tiles[pt][:, th * 1024 : (th + 1) * 1024], pst
            )

    def proj_v(wv_ch, th):
        """V token tiles tt = th*8 .. th*8+7."""
        x_ch = []
        for dc in range(NDC):
            xt = xw.tile([P, 1024], F32R, name=f"xv{th}_{dc}", tag="xt")
            nc.sync.dma_start(
                out=xt,
                in_=_r(d["xv_t"][dc * P : (dc + 1) * P, th * 1024 : (th + 1) * 1024]),
            )
            x_ch.append(xt)
        for tb in range(8):
            tt = th * 8 + tb
            pst = ps.tile([P, 1024], F32, name=f"psv{tt}", tag="qk")[:, :512]
            for dc in range(NDC):
                nc.tensor.matmul(
                    pst,
                    _r(x_ch[dc][:, tb * P : (tb + 1) * P]),
                    _r(wv_ch[dc]),
                    start=(dc == 0),
                    stop=(dc == NDC - 1 and not has_vb),
                )
            if has_vb:
                nc.tensor.matmul(
                    pst, _r(ones_row[:, :P]), _r(bias_sb["bv"]), start=False, stop=True
                )
            v3 = V[tt].rearrange("p (h c) -> p h c", c=DEP + 1)
            nc.vector.tensor_copy(v3[:, :, 0:DEP], pst)
            # ones column (col h*65+64) from a DRAM constant
            nc.sync.dma_start(out=v3[:, :, DEP : DEP + 1], in_=_r(d["ones_c"][:, :]))

    def attention(hp, qb, OT_tile):
        """Heads A=2hp (rows 0-63), B=2hp+1 (rows 64-127); queries
        qb*1024..(qb+1)*1024. Writes OT_tile[128, 1024]."""
        A, B = 2 * hp, 2 * hp + 1
        halves = ((A, 0), (B, DEP))
        pv = {}
        for hh, _ in halves:
            for qs in range(2):
                pv[(hh, qs)] = pvps.tile(
                    [DEP + 1, 512], F32, name=f"pv{qb}_{hh}_{qs}", tag="pv"
                )
        for kt in range(NTT):
            qk = {
                hh: ps.tile([P, 1024], F32, name=f"qk{qb}_{hh}_{kt}", tag="qk")
                for hh, _ in halves
            }
            for hh, r0 in halves:
                lhsT = _r(KT[hp][r0 : r0 + DEP, kt * P : (kt + 1) * P])
                for qs in range(2):
                    q0 = qb * 1024 + qs * 512
                    nc.tensor.matmul(
                        qk[hh][:, qs * 512 : (qs + 1) * 512],
                        lhsT,
                        _r(QT[hp][r0 : r0 + DEP, q0 : q0 + 512]),
                        start=True,
                        stop=True,
                    )
            es = {}
            for hh, _ in halves:
                es[hh] = espool.tile(
                    [P, 1024], F32R, name=f"es{qb}_{hh}_{kt}", tag="es"
                )
                nc.scalar.activation(es[hh], qk[hh], EXP, scale=0.125)
            for hh, _ in halves:
                lhsT = _r(V[kt][:, hh * (DEP + 1) : (hh + 1) * (DEP + 1)])
                for qs in range(2):
                    nc.tensor.matmul(
                        pv[(hh, qs)],
                        lhsT,
                        _r(es[hh][:, qs * 512 : (qs + 1) * 512]),
                        start=(kt == 0),
                        stop=(kt == NTT - 1),
                    )
        # finalize: batched reciprocal of the 4 denominator rows (placed at
        # partitions 0/32/64/96 -- other bases are rejected)
        order = [(A, 0), (A, 1), (B, 0), (B, 1)]
        den = rcpool.tile([128, 512], F32, name=f"den{qb}_{hp}", tag="den")
        nc.vector.memset(den, 1.0)
        for j, (hh, qs) in enumerate(order):
            nc.vector.tensor_copy(
                den[32 * j : 32 * j + 1, :], pv[(hh, qs)][DEP : DEP + 1, :]
            )
        rec = rcpool.tile([128, 512], F32, name=f"rec{qb}_{hp}", tag="rec")
        nc.vector.reciprocal(rec, den)
        for j, (hh, qs) in enumerate(order):
            bct = bcpool.tile([DEP, 512], F32, name=f"bct{qb}_{hh}_{qs}", tag="bct")
            nc.gpsimd.partition_broadcast(bct, rec[32 * j : 32 * j + 1, :])
            r0 = (hh % 2) * DEP
            nc.vector.tensor_mul(
                OT_tile[r0 : r0 + DEP, qs * 512 : (qs + 1) * 512],
                pv[(hh, qs)][0:DEP, :],
                bct,
            )

    # ---------------- emission (priority) order --------------------------
    # K proj (all tokens) -> Q proj half 0 -> attention qb=0 -> V proj ->
    # Q proj half 1 -> attention qb=1 -> dense.  Attention qb=0 is emitted
    # before V-proj/Q-h1 only in the sense that V/Q-h1 land after it in
    # priority; tile-granular deps let the scheduler fill PE stalls.
    OT_all = {}

    wk_ch = load_w("wk_t")
    for th in range(2):
        proj_qk(wk_ch, "xk_t", "bk", KT, th)
    wq_ch = load_w("wq_t", 0)
    proj_qk(wq_ch, "xq_t", "bq", QT, 0)

    wv_ch = load_w("wv_t")
    for th in range(2):
        proj_v(wv_ch, th)

    for hp in range(NPT):
        OT_all[(hp, 0)] = otp.tile([P, 1024], F32R, name=f"OT_{hp}_0", tag="ot")
        attention(hp, 0, OT_all[(hp, 0)])

    wq_ch = load_w("wq_t", 1)
    proj_qk(wq_ch, "xq_t", "bq", QT, 1)

    for hp in range(NPT):
        OT_all[(hp, 1)] = otp.tile([P, 1024], F32R, name=f"OT_{hp}_1", tag="ot")
        attention(hp, 1, OT_all[(hp, 1)])

    # ---- dense ----------------------------------------------------------
    # release attention PSUM pools; dense gets a deep pool over all 8 banks
    attn_psum_ctx.close()
    dsps = ctx.enter_context(tc.tile_pool(name="dsps", bufs=4, space="PSUM"))
    ds_ch = {}
    for pt in range(NPT):
        for nt in range(2):
            wt = wpool.tile(
                [P, PD], F32R, name=f"ds{pt}_{nt}", tag=f"w{pt * 2 + nt}"
            )
            nc.sync.dma_start(
                out=wt,
                in_=_r(d["ds_t"][pt * P : (pt + 1) * P, nt * 512 : (nt + 1) * 512]),
            )
            ds_ch[(pt, nt)] = wt

    for tt in range(NTT):
        qh, tb = tt // 8, tt % 8
        pst = dsps.tile([P, 1024], F32, name=f"psd{tt}", tag="dqk")
        for pt in range(NPT):
            lhsT = _r(OT_all[(pt, qh)][:, tb * P : (tb + 1) * P])
            for nt in range(2):
                nc.tensor.matmul(
                    pst[:, nt * 512 : (nt + 1) * 512],
                    lhsT,
                    _r(ds_ch[(pt, nt)]),
                    start=(pt == 0),
                    stop=(pt == NPT - 1),
                )
        for nt in range(2):
            osb = outpool.tile([P, 512], F32, name=f"osb{tt}_{nt}", tag="osb")
            nc.vector.tensor_copy(osb, pst[:, nt * 512 : (nt + 1) * 512])
            nc.sync.dma_start(
                out=d["out"][tt * P : (tt + 1) * P, nt * 512 : (nt + 1) * 512],
                in_=osb,
            )


def build_nc(with_bias=(False, False, False)):
    nc = bacc.Bacc("TRN2", target_bir_lowering=False, debug=False)
    d = {}
    for name, shape in (
        ("xq_t", [DM, S]),
        ("xk_t", [DM, S]),
        ("xv_t", [DM, S]),
        ("wq_t", [DM, PD]),
        ("wk_t", [DM, PD]),
        ("wv_t", [DM, PD]),
        ("ds_t", [PD, DM]),
        ("ones_c", [P, NH]),
    ):
        d[name] = nc.dram_tensor(name, shape, F32, kind="ExternalInput").ap()
    if any(with_bias):
        d["ones_r"] = nc.dram_tensor(
            "ones_r", [1, 1024], F32, kind="ExternalInput"
        ).ap()
    for flag, nm in zip(with_bias, ("bq", "bk", "bv")):
        if flag:
            d[nm] = nc.dram_tensor(nm, [1, PD], F32, kind="ExternalInput").ap()
    d["out"] = nc.dram_tensor("out", [S, DM], F32, kind="ExternalOutput").ap()

    with tile.TileContext(nc) as tc:
        with ExitStack() as ctx:
            _emit(nc, tc, ctx, d, with_bias)
    nc.compile()
    return nc


_CACHE = {}


def _get_nc(with_bias):
    if with_bias not in _CACHE:
        _CACHE[with_bias] = build_nc(with_bias)
    return _CACHE[with_bias]


def make_in_maps(query, key, value, wq_w, wq_b, wk_w, wk_b, wv_w, wv_b, dense_w):
    """Host-side sharding: 8 in_maps for cores (b, g)."""
    with_bias = (
        bool(np.any(wq_b)),
        bool(np.any(wk_b)),
        bool(np.any(wv_b)),
    )
    c = np.ascontiguousarray
    in_maps = []
    for core in range(8):
        b, g = core // 2, core % 2
        sl = slice(g * PD, (g + 1) * PD)
        m = {
            "xq_t": c(query[b].T),
            "xk_t": c(key[b].T),
            "xv_t": c(value[b].T),
            "wq_t": c(wq_w[sl].T),
            "wk_t": c(wk_w[sl].T),
            "wv_t": c(wv_w[sl].T),
            "ds_t": c(dense_w[:, sl].T),
            "ones_c": np.ones((P, NH), np.float32),
        }
        if any(with_bias):
            m["ones_r"] = np.ones((1, 1024), np.float32)
        if with_bias[0]:
            m["bq"] = c(wq_b[sl][None, :])
        if with_bias[1]:
            m["bk"] = c(wk_b[sl][None, :])
        if with_bias[2]:
            m["bv"] = c(wv_b[sl][None, :])
        in_maps.append(m)
    return in_maps, with_bias


def kernel(
    query, key, value, wq_w, wq_b, wk_w, wk_b, wv_w, wv_b, dense_w, dense_b, **kw
):
    query = np.asarray(query, np.float32)
    key = np.asarray(key, np.float32)
    value = np.asarray(value, np.float32)
    in_maps, with_bias = make_in_maps(
        query, key, value,
        np.asarray(wq_w, np.float32), np.asarray(wq_b, np.float32),
        np.asarray(wk_w, np.float32), np.asarray(wk_b, np.float32),
        np.asarray(wv_w, np.float32), np.asarray(wv_b, np.float32),
        np.asarray(dense_w, np.float32),
    )
    nc = _get_nc(with_bias)
    res = run_bass_kernel_spmd(nc, in_maps, core_ids=list(range(8)))
    B = query.shape[0]
    out = np.empty((B, S, DM), np.float32)
    db = np.asarray(dense_b, np.float32)
    for b in range(B):
        out[b] = res.results[2 * b]["out"] + res.results[2 * b + 1]["out"] + db
    return out
